# revision 11
# baseline (speedup 1.0000x reference)
"""Trainium2 Bass kernel for nn_Decay2D (decay-masked linear attention).

Math: the reference's Hillis-Steele scan with decay-squaring order composes
to coefficient d^ceil((t-s)/2) on store[s] = scale*k_s v_s^T, so

    out[t] = scale^2 * sum_{s<=t} d^ceil((t-s)/2) (q_t . k_s) v_s  @ Wo^T

computed as chunked linear attention with two [K,V] carry states (even/odd
decay chains), never materializing the [B,T,K,V] memory.

Sharding: 8 cores = 4 batches x 2 sequence halves. Each core builds the
carry state over its prefix rows and runs full attention + output projection
for its own 1024 rows.

Implementation notes:
- bf16 on the PE, fp32 PSUM accumulation, fp32 carry states.
- Projection biases ride the sigmoid activation's bias AP (no bias matmuls).
- The PE instruction stream is software-pipelined across the two sequence
  groups (g1 transposes/scores overlap g0's state chain and output copies)
  to keep the PE continuously busy and hold the high p-state clock.
- Output is written bf16 (fro tolerance 2e-2) and widened on host; the two
  po halves land in one 2-bank PSUM tile so each chunk needs one wide copy.
"""

from contextlib import ExitStack

import numpy as np

import concourse.bass as bass
import concourse.bacc as bacc
import concourse.mybir as mybir
import concourse.tile as tile
from concourse import bass_utils
from concourse.alu_op_type import AluOpType
from concourse.bass import ts

F32 = mybir.dt.float32
BF16 = mybir.dt.bfloat16
SIG = mybir.ActivationFunctionType.Sigmoid

B, T, E, K, V = 4, 2048, 1024, 64, 64
DECAY = 0.9
C = 128          # chunk length
HT = T // 2      # rows per core (sequence half)
NCH = HT // C    # chunks per half (8)
NEC = E // 128   # embed sub-chunks (8)
GW = 512         # group width: 4 chunks per PSUM bank
GCH = GW // C    # chunks per group (4)
NG = HT // GW    # groups per half (2)
DC2 = float(DECAY ** (C // 2))
N_CORES = 8

# packed-constants layout (masks only; projection weights travel separately)
def _mklayout(regions):
    out, off = {}, 0
    for n, r, c in regions:
        out[n] = (r, off, c)
        off += c
    return out, off


_HOT, HOT_W = _mklayout([
    ("ident", 64, 64), ("identhi", 128, 64),
    ("mlocT4", C, GW), ("cemat", K, GW), ("comat", K, GW),
    ("wgeo_p1", C, GCH * 2 * K), ("wgeo_p2", C, GCH * 2 * K),
])
HW2 = GW // 2   # half-group width
PRE = 256       # truncated prefix length (2 chunks; older rows decay < 1e-12)
PCH = PRE // C  # prefix chunks (2)

# gamma columns: 0 prefix flag, 1 bk|bv, 2 bq
G_GAMMA, G_BKV, G_BQ = 0, 1, 2


def _host_constants():
    d = DECAY
    scale2 = 1.0 - d
    i = np.arange(C)
    j = np.arange(C)
    delta = i[:, None] - j[None, :]
    # intra-chunk decay mask, transposed to [tcol(j), trow(i)], scale^2 folded
    mloc = np.where(delta >= 0, d ** np.ceil(delta / 2.0), 0.0) * scale2
    mlocT4 = np.tile(np.ascontiguousarray(mloc.T), (1, GCH)).astype(np.float32)
    # boundary coefficient per local row i (scale^2 folded), split by parity
    c = d ** np.ceil((i + 1) / 2.0) * scale2
    ce = np.where(i % 2 == 0, c, 0.0).astype(np.float32)
    co = np.where(i % 2 == 1, c, 0.0).astype(np.float32)
    cemat = np.tile(np.broadcast_to(ce, (K, C)), (1, GCH)).astype(np.float32)
    comat = np.tile(np.broadcast_to(co, (K, C)), (1, GCH)).astype(np.float32)
    # state-update row weights (per t within chunk)
    u_o = np.where(j % 2 == 1, d ** ((C - 1 - j) / 2.0), 0.0)
    u_e = np.where(j % 2 == 0, d ** ((C - 2 - j) / 2.0), 0.0)
    wge = (u_o + u_e).astype(np.float32)[:, None]          # [C,1]
    wgo = (u_o + d * u_e).astype(np.float32)[:, None]

    def wmat_stacked(paired):
        # [C, GCH*2K]: per chunk block [wge*K | wgo*K]; paired mode folds an
        # extra dC2 into the first chunk of each pair
        blocks = []
        for cl in range(GCH):
            s = DC2 if (paired and cl % 2 == 0) else 1.0
            blocks.append(np.repeat(np.concatenate([wge * s, wgo * s], 1), K, 1))
        return np.concatenate(blocks, 1).astype(np.float32)

    return {
        "mlocT4": mlocT4,
        "cemat": np.ascontiguousarray(cemat),
        "comat": np.ascontiguousarray(comat),
        "wgeo_p1": wmat_stacked(True),
        "wgeo_p2": wmat_stacked(False),
        "ident64": np.eye(64, dtype=np.float32),
    }


def _build_program(has_bv):
    nc = bacc.Bacc(
        "TRN2",
        debug=False,
        enable_asserts=False,
        target_bir_lowering=False,
        num_devices=N_CORES,
    )

    def din(name, shape, dtype=F32):
        return nc.dram_tensor(name, shape, dtype, kind="ExternalInput").ap()

    # x pre-packed on host: per group one contiguous [128, NEC*GW] block,
    # free layout (ec, half, t). Prefix truncated to its last PRE rows.
    xpre2 = din("xpre2", [128, NEC * PRE], BF16)
    wproj_d = din("wproj", [128, NEC * 3 * K], BF16)
    xq2 = din("xq2", [NG * 128, NEC * GW], BF16)
    chot_d = din("chot", [128, HOT_W], BF16)
    woT_d = din("woT", [V, E], BF16)
    gamma_d = din("gamma_col", [128, 3])
    out_d = nc.dram_tensor("out", [HT, E], BF16, kind="ExternalOutput").ap()

    with ExitStack() as ctx:
        tc = ctx.enter_context(tile.TileContext(nc))

        consts = ctx.enter_context(tc.tile_pool(name="consts", bufs=1))
        state = ctx.enter_context(tc.tile_pool(name="state", bufs=1))
        xpool = ctx.enter_context(tc.tile_pool(name="xg", bufs=4))
        spool = ctx.enter_context(tc.tile_pool(name="sml", bufs=2))
        opool = ctx.enter_context(tc.tile_pool(name="osb", bufs=3))
        pbig = ctx.enter_context(tc.tile_pool(name="pbig", bufs=2, space="PSUM"))
        psml = ctx.enter_context(tc.tile_pool(name="psml", bufs=2, space="PSUM"))
        pattn = ctx.enter_context(tc.tile_pool(name="pattn", bufs=1, space="PSUM"))

        # ---- loads (issue order = queue order on sync) ----
        def ld(pool, shape, dtype, src, name):
            t = pool.tile(shape, dtype, name=name)
            nc.sync.dma_start(t[:], src)
            return t

        gamma = ld(consts, [128, 3], F32, gamma_d[:], "gamma_sb")
        wproj = ld(consts, [128, NEC * 3 * K], BF16, wproj_d[:], "wproj")
        xp = xpool.tile([128, NEC * PRE], BF16, tag="xp", name="xp", bufs=1)
        nc.sync.dma_start(xp[:], xpre2[:])

        def load_xgroup(g, name):
            xg = xpool.tile([128, NEC * GW], BF16, tag="xg", name=name)
            nc.sync.dma_start(xg[:], xq2[g * 128 : (g + 1) * 128, :])
            return xg

        xg2_0 = load_xgroup(0, "xg2_0")
        xg2_1 = load_xgroup(1, "xg2_1")
        xg2s = [xg2_0, xg2_1]
        chot = ld(consts, [128, HOT_W], BF16, chot_d[:], "chot")
        wo = ld(consts, [V, E], BF16, woT_d[:], "wo")

        def reg(pack, layout, name):
            r, o, c = layout[name]
            return pack[0:r, o : o + c]

        wkv = wproj[:, 0 : NEC * 2 * K]
        wq = wproj[:, NEC * 2 * K : NEC * 3 * K]
        ident, identhi = reg(chot, _HOT, "ident"), reg(chot, _HOT, "identhi")
        mlocT4 = reg(chot, _HOT, "mlocT4")
        cemat, comat = reg(chot, _HOT, "cemat"), reg(chot, _HOT, "comat")
        wgeo_p1, wgeo_p2 = reg(chot, _HOT, "wgeo_p1"), reg(chot, _HOT, "wgeo_p2")
        bk_ap = gamma[0:K, G_BKV : G_BKV + 1]
        bv_ap = gamma[K : 2 * K, G_BKV : G_BKV + 1]
        bq_ap = gamma[0:K, G_BQ : G_BQ + 1]

        qT_all = consts.tile([K, HT], BF16, name="qT_all")
        kT_all = consts.tile([K, HT], BF16, name="kT_all")
        lt_all = consts.tile([V, HT], BF16, name="lt_all")
        geo_all = state.tile([2 * K, NCH * V], F32, name="geo_all")
        geo_bf = state.tile([2 * K, NCH * V], BF16, name="geo_bf")

        # ---- stage helpers ----
        def proj(w, rows, xg, cols, name):
            p = pbig.tile([rows, cols], F32, tag="pB", name=name)
            for ec in range(NEC):
                nc.tensor.matmul(p[:], w[:, ts(ec, rows)], xg[:, ts(ec, cols)],
                                 start=(ec == 0), stop=(ec == NEC - 1))
            return p

        def transposes(kT_src, vT_src, nch, tagp):
            pkn = psml.tile([C, nch * 2 * K], BF16, tag="pS", name=f"pkn{tagp}")
            for cl in range(nch):
                kT_i = kT_src[:, ts(cl, C)]
                nc.tensor.matmul(pkn[:, cl * 2 * K : cl * 2 * K + K],
                                 kT_i, ident[:], is_transpose=True)
                nc.tensor.matmul(pkn[:, cl * 2 * K + K : (cl + 1) * 2 * K],
                                 kT_i, ident[:], is_transpose=True)
            pvn = psml.tile([C, nch * V], BF16, tag="pS", name=f"pvn{tagp}")
            for cl in range(nch):
                nc.tensor.matmul(pvn[:, ts(cl, V)], vT_src[K : 2 * K, ts(cl, C)],
                                 identhi[K : 2 * K, :], is_transpose=True)
            kn = spool.tile([C, nch * 2 * K], BF16, tag=f"kn{tagp}", name=f"kn{tagp}")
            nc.scalar.copy(kn[:], pkn[:])
            v_b = spool.tile([C, nch * V], BF16, tag=f"v{tagp}", name=f"v{tagp}")
            nc.vector.tensor_copy(v_b[:], pvn[:])
            return kn, v_b

        def scores_stage(g):
            ps = pbig.tile([C, GW], F32, tag="pB", name="ps")
            for cl in range(GCH):
                i = g * GCH + cl
                nc.tensor.matmul(ps[:, ts(cl, C)], kT_all[:, ts(i, C)],
                                 qT_all[:, ts(i, C)], start=True, stop=True)
            sT_b = spool.tile([C, GW], BF16, tag=f"sm{g}", name="sT_b")
            nc.vector.tensor_mul(sT_b[:], ps[:], mlocT4[:])
            qTeo = spool.tile([2 * K, GW], BF16, tag=f"qeo{g}", name="qTeo")
            nc.vector.tensor_mul(qTeo[0:K, :], qT_all[:, ts(g, GW)], cemat[:])
            nc.gpsimd.tensor_mul(qTeo[K : 2 * K, :], qT_all[:, ts(g, GW)], comat[:])
            return sT_b, qTeo

        def kgeo_stage(kn, width, wsel, tagp):
            kgeo = spool.tile([C, width], BF16, tag=f"kg{tagp}", name=f"kgeo{tagp}")
            nc.vector.tensor_mul(kgeo[:], kn[:], wsel[:, :width])
            return kgeo

        def attn_stage(g, v_b, sT_b, qTeo):
            plt = pattn.tile([V, GW], F32, tag=f"pLT{g}", name="plt")
            for cl in range(GCH):
                i = g * GCH + cl
                dst = plt[:, ts(cl, C)]
                nc.tensor.matmul(dst, v_b[:, ts(cl, V)], sT_b[:, ts(cl, C)],
                                 start=True, stop=False)
                nc.tensor.matmul(dst, geo_bf[:, ts(i, V)], qTeo[:, ts(cl, C)],
                                 start=False, stop=True)
            if g == 0:
                nc.scalar.copy(lt_all[:, ts(g, GW)], plt[:])
            else:
                nc.vector.tensor_copy(lt_all[:, ts(g, GW)], plt[:])

        def out_stage(g):
            for cl in range(GCH):
                i = g * GCH + cl
                out_sb = opool.tile([C, E], BF16, tag="osb", name="out_sb")
                po = psml.tile([C, E], F32, tag="pS", name="po")
                for h in range(2):
                    nc.tensor.matmul(po[:, ts(h, GW)], lt_all[:, ts(i, C)],
                                     wo[:, ts(h, GW)], start=True, stop=True)
                nc.scalar.copy(out_sb[:, 0:GW], po[:, 0:GW])
                nc.vector.tensor_copy(out_sb[:, GW:E], po[:, GW:E])
                nc.gpsimd.dma_start(out_d[ts(i, C), :], out_sb[:])

        def chain_steps(lo, hi, pu2):
            # geo_i = dC2 * geo_{i-1} + pu2_{i-1}, bf16 copy per chunk
            for i in range(lo, hi):
                nc.vector.scalar_tensor_tensor(
                    geo_all[:, ts(i, V)], geo_all[:, ts(i - 1, V)], DC2,
                    pu2[:, ts(i - 1, V)], AluOpType.mult, AluOpType.add,
                )
                nc.gpsimd.tensor_copy(geo_bf[:, ts(i, V)], geo_all[:, ts(i, V)])

        # ============ projections (PE dense from the start) ============
        pkv1 = proj(wkv, 2 * K, xp, PRE, "pkv1")
        kT1 = spool.tile([K, PRE], BF16, tag="kT1", name="kT1")
        nc.scalar.activation(kT1[:], pkv1[0:K, :], SIG, bias=bk_ap)
        vT1 = spool.tile([2 * K, PRE], BF16, tag="vT1", name="vT1")
        nc.scalar.copy(vT1[K : 2 * K, :], pkv1[K : 2 * K, :])
        if has_bv:
            nc.vector.tensor_scalar_add(vT1[K : 2 * K, :], vT1[K : 2 * K, :], bv_ap)

        vT_sbs = []
        for g in range(NG):
            pkv = proj(wkv, 2 * K, xg2s[g], GW, f"pkv_{g}")
            nc.scalar.activation(kT_all[:, ts(g, GW)], pkv[0:K, :], SIG, bias=bk_ap)
            vT_sb = spool.tile([2 * K, GW], BF16, tag=f"vT{g}", name=f"vT_sb{g}")
            nc.scalar.copy(vT_sb[K : 2 * K, :], pkv[K : 2 * K, :])
            if has_bv:
                nc.vector.tensor_scalar_add(
                    vT_sb[K : 2 * K, :], vT_sb[K : 2 * K, :], bv_ap)
            vT_sbs.append(vT_sb)
            pg = proj(wq, K, xg2s[g], GW, f"pg_{g}")
            nc.scalar.activation(qT_all[:, ts(g, GW)], pg[:], SIG, bias=bq_ap)

        # ============ pipelined groups ============
        # prefix + g0 transposes
        kn1, v1_b = transposes(kT1, vT1, PCH, "1")
        kn0, v0_b = transposes(kT_all[:, 0:GW], vT_sbs[0], GCH, "20")
        kgeo1 = kgeo_stage(kn1, PCH * 2 * K, wgeo_p1, "1")

        # prefix state: geo1 = dC2*U_c0 + U_c1 (dC2 in weights)
        pu1 = pbig.tile([2 * K, V], F32, tag="pB", name="pu1")
        nc.tensor.matmul(pu1[:], kgeo1[:, 0 : 2 * K], v1_b[:, 0:V],
                         start=True, stop=False)
        nc.tensor.matmul(pu1[:], kgeo1[:, 2 * K : 4 * K], v1_b[:, V : 2 * V],
                         start=False, stop=True)
        geo1 = state.tile([2 * K, V], F32, name="geo1")
        nc.vector.tensor_copy(geo1[:], pu1[:])

        # g0 scores + weighted keys, then g0 states (chain 1..4)
        sT_0, qTeo_0 = scores_stage(0)
        kgeo_0 = kgeo_stage(kn0, GCH * 2 * K, wgeo_p2, "20")
        pu2 = pbig.tile([2 * K, (NCH - 1) * V], F32, tag="pB", name="pu2")
        for i in range(GCH):
            nc.tensor.matmul(pu2[:, ts(i, V)], kgeo_0[:, ts(i, 2 * K)],
                             v0_b[:, ts(i, V)], start=True, stop=True)
        nc.vector.tensor_scalar_mul(geo_all[:, 0:V], geo1[:],
                                    gamma[:, G_GAMMA : G_GAMMA + 1])
        nc.gpsimd.tensor_copy(geo_bf[:, 0:V], geo_all[:, 0:V])
        chain_steps(1, GCH + 1, pu2)

        # g1 transposes overlap the g0 chain / copies
        kn1g, v1g_b = transposes(kT_all[:, GW : 2 * GW], vT_sbs[1], GCH, "21")

        # g0 attention, g1 scores, g0 output
        attn_stage(0, v0_b, sT_0, qTeo_0)
        sT_1, qTeo_1 = scores_stage(1)
        out_stage(0)

        # g1 states (chain 5..7), attention, output
        kgeo_1 = kgeo_stage(kn1g, GCH * 2 * K, wgeo_p2, "21")
        for i in range(GCH, NCH - 1):
            cl = i - GCH
            nc.tensor.matmul(pu2[:, ts(i, V)], kgeo_1[:, ts(cl, 2 * K)],
                             v1g_b[:, ts(cl, V)], start=True, stop=True)
        chain_steps(GCH + 1, NCH, pu2)
        attn_stage(1, v1g_b, sT_1, qTeo_1)
        out_stage(1)

    nc.compile()
    return nc


_CACHE = {}


def _get_program(has_bv):
    key = ("nc", has_bv)
    if key not in _CACHE:
        _CACHE[key] = _build_program(has_bv)
    return _CACHE[key]


def _make_in_maps(x, Wk, bk, Wv, bv, Wq, bq, Wo):
    import ml_dtypes

    bfd = ml_dtypes.bfloat16
    consts = _host_constants()

    def pack2(Wa, Wb):
        # [128, NEC*(outA+outB)]: per embed sub-chunk, [Wa_ec | Wb_ec] columns
        Wab = np.concatenate(
            [Wa.T.reshape(NEC, 128, -1), Wb.T.reshape(NEC, 128, -1)], 2
        )
        return np.ascontiguousarray(
            Wab.transpose(1, 0, 2).reshape(128, -1)
        ).astype(bfd)

    def pack1(W):
        return np.ascontiguousarray(
            W.T.reshape(NEC, 128, -1).transpose(1, 0, 2).reshape(128, -1)
        ).astype(bfd)

    identhi = np.zeros((128, 64), np.float32)
    identhi[64:128, :] = np.eye(64)
    chot = np.zeros((128, HOT_W), np.float32)

    def setreg(pack, layout, name, arr):
        r, o, c = layout[name]
        pack[0:r, o : o + c] = arr

    setreg(chot, _HOT, "ident", consts["ident64"])
    setreg(chot, _HOT, "identhi", identhi)
    setreg(chot, _HOT, "mlocT4", consts["mlocT4"])
    setreg(chot, _HOT, "cemat", consts["cemat"])
    setreg(chot, _HOT, "comat", consts["comat"])
    setreg(chot, _HOT, "wgeo_p1", consts["wgeo_p1"])
    setreg(chot, _HOT, "wgeo_p2", consts["wgeo_p2"])

    shared = {
        "wproj": np.concatenate([pack2(Wk, Wv), pack1(Wq)], 1),
        "chot": chot.astype(bfd),
        "woT": np.ascontiguousarray(Wo.T).astype(bfd),
    }

    def pack_x(xh):
        # [E, HT] -> [NG*128, (ec, half, t)]: one contiguous block per group
        v = xh.reshape(NEC, 128, NG, 2, HW2).transpose(2, 1, 0, 3, 4)
        return np.ascontiguousarray(v.reshape(NG * 128, NEC * GW)).astype(bfd)

    def pack_pre(xh):
        # last PRE prefix rows -> [128, (ec, t)] contiguous block
        v = xh[:, HT - PRE :].reshape(NEC, 128, PRE).transpose(1, 0, 2)
        return np.ascontiguousarray(v.reshape(128, NEC * PRE)).astype(bfd)

    gcol = np.zeros((128, 3), np.float32)
    gcol[0:K, G_BKV] = bk
    gcol[K : 2 * K, G_BKV] = bv
    gcol[0:K, G_BQ] = bq

    zeros_pre = np.zeros((128, NEC * PRE), bfd)
    in_maps = []
    for c in range(N_CORES):
        b, h = c // 2, c % 2
        xbT = np.ascontiguousarray(x[b].T)  # [E, T]
        m = dict(shared)
        m["xpre2"] = pack_pre(xbT[:, :HT]) if h == 1 else zeros_pre
        m["xq2"] = pack_x(xbT[:, h * HT : (h + 1) * HT])
        g = gcol.copy()
        g[:, G_GAMMA] = float(h)
        m["gamma_col"] = g
        in_maps.append(m)
    return in_maps


def run(inputs, trace=False):
    """Run on 8 cores; returns (output, BassKernelResults)."""
    inp = {k: np.asarray(v) for k, v in inputs.items()}
    has_bv = bool(np.any(inp["bv"]))
    nc = _get_program(has_bv)
    in_maps = _make_in_maps(**inp)
    res = bass_utils.run_bass_kernel_spmd(
        nc, in_maps, core_ids=list(range(N_CORES)), trace=trace
    )
    out = np.empty((B, T, E), np.float32)
    for c in range(N_CORES):
        b, h = c // 2, c % 2
        out[b, h * HT : (h + 1) * HT, :] = res.results[c]["out"].astype(np.float32)
    return out, res


def kernel(**inputs):
    out, _ = run(inputs, trace=False)
    return out


# revision 12
# speedup vs baseline: 1.0846x; 1.0846x over previous
"""Trainium2 Bass kernel for nn_Decay2D (decay-masked linear attention).

Math: the reference's Hillis-Steele scan with decay-squaring order composes
to coefficient d^ceil((t-s)/2) on store[s] = scale*k_s v_s^T, so

    out[t] = scale^2 * sum_{s<=t} d^ceil((t-s)/2) (q_t . k_s) v_s  @ Wo^T

computed as chunked linear attention with two [K,V] carry states (even/odd
decay chains), never materializing the [B,T,K,V] memory.

Sharding: 8 cores = 4 batches x 2 sequence halves. Each core builds the
carry state over its prefix rows and runs full attention + output projection
for its own 1024 rows.

Implementation notes:
- bf16 on the PE, fp32 PSUM accumulation, fp32 carry states.
- Projection biases ride the sigmoid activation's bias AP (no bias matmuls).
- The PE instruction stream is software-pipelined across the two sequence
  groups (g1 transposes/scores overlap g0's state chain and output copies)
  to keep the PE continuously busy and hold the high p-state clock.
- Output is written bf16 (fro tolerance 2e-2) and widened on host; the two
  po halves land in one 2-bank PSUM tile so each chunk needs one wide copy.
"""

from contextlib import ExitStack

import numpy as np

import concourse.bass as bass
import concourse.bacc as bacc
import concourse.mybir as mybir
import concourse.tile as tile
from concourse import bass_utils
from concourse.alu_op_type import AluOpType
from concourse.bass import ts

F32 = mybir.dt.float32
BF16 = mybir.dt.bfloat16
SIG = mybir.ActivationFunctionType.Sigmoid

B, T, E, K, V = 4, 2048, 1024, 64, 64
DECAY = 0.9
C = 128          # chunk length
HT = T // 2      # rows per core (sequence half)
NCH = HT // C    # chunks per half (8)
NEC = E // 128   # embed sub-chunks (8)
GW = 512         # group width: 4 chunks per PSUM bank
GCH = GW // C    # chunks per group (4)
NG = HT // GW    # groups per half (2)
DC2 = float(DECAY ** (C // 2))
N_CORES = 8

# packed-constants layout (masks only; projection weights travel separately)
def _mklayout(regions):
    out, off = {}, 0
    for n, r, c in regions:
        out[n] = (r, off, c)
        off += c
    return out, off


_HOT, HOT_W = _mklayout([
    ("ident", 64, 64), ("identhi", 128, 64),
    ("mlocT4", C, GW), ("cemat", K, GW), ("comat", K, GW),
    ("wgeo_p1", C, GCH * 2 * K), ("wgeo_p2", C, GCH * 2 * K),
])
HW2 = GW // 2   # half-group width
PRE = 256       # truncated prefix length (2 chunks; older rows decay < 1e-12)
PCH = PRE // C  # prefix chunks (2)

# gamma columns: 0 prefix flag, 1 bk|bv, 2 bq
G_GAMMA, G_BKV, G_BQ = 0, 1, 2


def _host_constants():
    d = DECAY
    scale2 = 1.0 - d
    i = np.arange(C)
    j = np.arange(C)
    delta = i[:, None] - j[None, :]
    # intra-chunk decay mask, transposed to [tcol(j), trow(i)], scale^2 folded
    mloc = np.where(delta >= 0, d ** np.ceil(delta / 2.0), 0.0) * scale2
    mlocT4 = np.tile(np.ascontiguousarray(mloc.T), (1, GCH)).astype(np.float32)
    # boundary coefficient per local row i (scale^2 folded), split by parity
    c = d ** np.ceil((i + 1) / 2.0) * scale2
    ce = np.where(i % 2 == 0, c, 0.0).astype(np.float32)
    co = np.where(i % 2 == 1, c, 0.0).astype(np.float32)
    cemat = np.tile(np.broadcast_to(ce, (K, C)), (1, GCH)).astype(np.float32)
    comat = np.tile(np.broadcast_to(co, (K, C)), (1, GCH)).astype(np.float32)
    # state-update row weights (per t within chunk)
    u_o = np.where(j % 2 == 1, d ** ((C - 1 - j) / 2.0), 0.0)
    u_e = np.where(j % 2 == 0, d ** ((C - 2 - j) / 2.0), 0.0)
    wge = (u_o + u_e).astype(np.float32)[:, None]          # [C,1]
    wgo = (u_o + d * u_e).astype(np.float32)[:, None]

    def wmat_stacked(paired):
        # [C, GCH*2K]: per chunk block [wge*K | wgo*K]; paired mode folds an
        # extra dC2 into the first chunk of each pair
        blocks = []
        for cl in range(GCH):
            s = DC2 if (paired and cl % 2 == 0) else 1.0
            blocks.append(np.repeat(np.concatenate([wge * s, wgo * s], 1), K, 1))
        return np.concatenate(blocks, 1).astype(np.float32)

    return {
        "mlocT4": mlocT4,
        "cemat": np.ascontiguousarray(cemat),
        "comat": np.ascontiguousarray(comat),
        "wgeo_p1": wmat_stacked(True),
        "wgeo_p2": wmat_stacked(False),
        "ident64": np.eye(64, dtype=np.float32),
    }


def _build_program(has_bv):
    nc = bacc.Bacc(
        "TRN2",
        debug=False,
        enable_asserts=False,
        target_bir_lowering=False,
        num_devices=N_CORES,
    )

    def din(name, shape, dtype=F32):
        return nc.dram_tensor(name, shape, dtype, kind="ExternalInput").ap()

    # x pre-packed on host: per group one contiguous [128, NEC*GW] block,
    # free layout (ec, half, t). Prefix truncated to its last PRE rows.
    xpre2 = din("xpre2", [128, NEC * PRE], BF16)
    wproj_d = din("wproj", [128, NEC * 3 * K], BF16)
    xq2 = din("xq2", [NG * 128, NEC * GW], BF16)
    chot_d = din("chot", [128, HOT_W], BF16)
    woT_d = din("woT", [V, E], BF16)
    gamma_d = din("gamma_col", [128, 3])
    out_d = nc.dram_tensor("out", [HT, E], BF16, kind="ExternalOutput").ap()

    with ExitStack() as ctx:
        tc = ctx.enter_context(tile.TileContext(nc))

        consts = ctx.enter_context(tc.tile_pool(name="consts", bufs=1))
        state = ctx.enter_context(tc.tile_pool(name="state", bufs=1))
        xpool = ctx.enter_context(tc.tile_pool(name="xg", bufs=4))
        spool = ctx.enter_context(tc.tile_pool(name="sml", bufs=2))
        opool = ctx.enter_context(tc.tile_pool(name="osb", bufs=3))
        pbig = ctx.enter_context(tc.tile_pool(name="pbig", bufs=2, space="PSUM"))
        psml = ctx.enter_context(tc.tile_pool(name="psml", bufs=2, space="PSUM"))
        pattn = ctx.enter_context(tc.tile_pool(name="pattn", bufs=2, space="PSUM"))

        # ---- loads (issue order = queue order on sync) ----
        def ld(pool, shape, dtype, src, name):
            t = pool.tile(shape, dtype, name=name)
            nc.sync.dma_start(t[:], src)
            return t

        gamma = ld(consts, [128, 3], F32, gamma_d[:], "gamma_sb")
        wproj = ld(consts, [128, NEC * 3 * K], BF16, wproj_d[:], "wproj")
        xp = xpool.tile([128, NEC * PRE], BF16, tag="xp", name="xp", bufs=1)
        nc.sync.dma_start(xp[:], xpre2[:])

        def load_xgroup(g, name):
            xg = xpool.tile([128, NEC * GW], BF16, tag="xg", name=name)
            nc.sync.dma_start(xg[:], xq2[g * 128 : (g + 1) * 128, :])
            return xg

        xg2_0 = load_xgroup(0, "xg2_0")
        xg2_1 = load_xgroup(1, "xg2_1")
        xg2s = [xg2_0, xg2_1]
        chot = ld(consts, [128, HOT_W], BF16, chot_d[:], "chot")
        wo = ld(consts, [V, E], BF16, woT_d[:], "wo")

        def reg(pack, layout, name):
            r, o, c = layout[name]
            return pack[0:r, o : o + c]

        wkv = wproj[:, 0 : NEC * 2 * K]
        wq = wproj[:, NEC * 2 * K : NEC * 3 * K]
        ident, identhi = reg(chot, _HOT, "ident"), reg(chot, _HOT, "identhi")
        mlocT4 = reg(chot, _HOT, "mlocT4")
        cemat, comat = reg(chot, _HOT, "cemat"), reg(chot, _HOT, "comat")
        wgeo_p1, wgeo_p2 = reg(chot, _HOT, "wgeo_p1"), reg(chot, _HOT, "wgeo_p2")
        bk_ap = gamma[0:K, G_BKV : G_BKV + 1]
        bv_ap = gamma[K : 2 * K, G_BKV : G_BKV + 1]
        bq_ap = gamma[0:K, G_BQ : G_BQ + 1]

        qT_all = consts.tile([K, HT], BF16, name="qT_all")
        kT_all = consts.tile([K, HT], BF16, name="kT_all")
        lt_all = consts.tile([V, HT], BF16, name="lt_all")
        geo_all = state.tile([2 * K, NCH * V], F32, name="geo_all")
        geo_bf = state.tile([2 * K, NCH * V], BF16, name="geo_bf")

        # ---- stage helpers ----
        def proj(w, rows, xg, cols, name):
            p = pbig.tile([rows, cols], F32, tag="pB", name=name)
            for ec in range(NEC):
                nc.tensor.matmul(p[:], w[:, ts(ec, rows)], xg[:, ts(ec, cols)],
                                 start=(ec == 0), stop=(ec == NEC - 1))
            return p

        def transposes(kT_src, vT_src, nch, tagp):
            pkn = psml.tile([C, nch * 2 * K], BF16, tag="pS", name=f"pkn{tagp}")
            for cl in range(nch):
                kT_i = kT_src[:, ts(cl, C)]
                nc.tensor.matmul(pkn[:, cl * 2 * K : cl * 2 * K + K],
                                 kT_i, ident[:], is_transpose=True)
                nc.tensor.matmul(pkn[:, cl * 2 * K + K : (cl + 1) * 2 * K],
                                 kT_i, ident[:], is_transpose=True)
            pvn = psml.tile([C, nch * V], BF16, tag="pS", name=f"pvn{tagp}")
            for cl in range(nch):
                nc.tensor.matmul(pvn[:, ts(cl, V)], vT_src[K : 2 * K, ts(cl, C)],
                                 identhi[K : 2 * K, :], is_transpose=True)
            kn = spool.tile([C, nch * 2 * K], BF16, tag=f"kn{tagp}", name=f"kn{tagp}")
            nc.scalar.copy(kn[:], pkn[:])
            v_b = spool.tile([C, nch * V], BF16, tag=f"v{tagp}", name=f"v{tagp}")
            nc.vector.tensor_copy(v_b[:], pvn[:])
            return kn, v_b

        def scores_stage(g):
            ps = pbig.tile([C, GW], F32, tag="pB", name="ps")
            for cl in range(GCH):
                i = g * GCH + cl
                nc.tensor.matmul(ps[:, ts(cl, C)], kT_all[:, ts(i, C)],
                                 qT_all[:, ts(i, C)], start=True, stop=True)
            sT_b = spool.tile([C, GW], BF16, tag=f"sm{g}", name="sT_b")
            nc.vector.tensor_mul(sT_b[:], ps[:], mlocT4[:])
            qTeo = spool.tile([2 * K, GW], BF16, tag=f"qeo{g}", name="qTeo")
            nc.vector.tensor_mul(qTeo[0:K, :], qT_all[:, ts(g, GW)], cemat[:])
            nc.gpsimd.tensor_mul(qTeo[K : 2 * K, :], qT_all[:, ts(g, GW)], comat[:])
            return sT_b, qTeo

        def kgeo_stage(kn, width, wsel, tagp):
            kgeo = spool.tile([C, width], BF16, tag=f"kg{tagp}", name=f"kgeo{tagp}")
            nc.vector.tensor_mul(kgeo[:], kn[:], wsel[:, :width])
            return kgeo

        def attn_stage(g, v_b, sT_b, qTeo):
            plt = pattn.tile([V, GW], F32, tag="pLT", name="plt")
            for cl in range(GCH):
                i = g * GCH + cl
                dst = plt[:, ts(cl, C)]
                nc.tensor.matmul(dst, v_b[:, ts(cl, V)], sT_b[:, ts(cl, C)],
                                 start=True, stop=False)
                nc.tensor.matmul(dst, geo_bf[:, ts(i, V)], qTeo[:, ts(cl, C)],
                                 start=False, stop=True)
                if cl % 2 == 0:
                    nc.scalar.copy(lt_all[:, ts(i, C)], dst)
                else:
                    nc.vector.tensor_copy(lt_all[:, ts(i, C)], dst)

        def out_stage(g):
            for cl in range(GCH):
                i = g * GCH + cl
                out_sb = opool.tile([C, E], BF16, tag="osb", name="out_sb")
                for h in range(2):
                    po = pbig.tile([C, GW], F32, tag="pB", name="po")
                    nc.tensor.matmul(po[:], lt_all[:, ts(i, C)],
                                     wo[:, ts(h, GW)], start=True, stop=True)
                    if h == 0:
                        nc.scalar.copy(out_sb[:, ts(h, GW)], po[:])
                    else:
                        nc.vector.tensor_copy(out_sb[:, ts(h, GW)], po[:])
                nc.sync.dma_start(out_d[ts(i, C), :], out_sb[:])

        def chain_steps(lo, hi, pu2):
            # geo_i = dC2 * geo_{i-1} + pu2_{i-1}, bf16 copy per chunk
            for i in range(lo, hi):
                nc.vector.scalar_tensor_tensor(
                    geo_all[:, ts(i, V)], geo_all[:, ts(i - 1, V)], DC2,
                    pu2[:, ts(i - 1, V)], AluOpType.mult, AluOpType.add,
                )
                nc.gpsimd.tensor_copy(geo_bf[:, ts(i, V)], geo_all[:, ts(i, V)])

        # ============ projections (PE dense from the start) ============
        pkv1 = proj(wkv, 2 * K, xp, PRE, "pkv1")
        kT1 = spool.tile([K, PRE], BF16, tag="kT1", name="kT1")
        nc.scalar.activation(kT1[:], pkv1[0:K, :], SIG, bias=bk_ap)
        vT1 = spool.tile([2 * K, PRE], BF16, tag="vT1", name="vT1")
        nc.scalar.copy(vT1[K : 2 * K, :], pkv1[K : 2 * K, :])
        if has_bv:
            nc.vector.tensor_scalar_add(vT1[K : 2 * K, :], vT1[K : 2 * K, :], bv_ap)

        vT_sbs = []
        for g in range(NG):
            pkv = proj(wkv, 2 * K, xg2s[g], GW, f"pkv_{g}")
            nc.scalar.activation(kT_all[:, ts(g, GW)], pkv[0:K, :], SIG, bias=bk_ap)
            vT_sb = spool.tile([2 * K, GW], BF16, tag=f"vT{g}", name=f"vT_sb{g}")
            nc.scalar.copy(vT_sb[K : 2 * K, :], pkv[K : 2 * K, :])
            if has_bv:
                nc.vector.tensor_scalar_add(
                    vT_sb[K : 2 * K, :], vT_sb[K : 2 * K, :], bv_ap)
            vT_sbs.append(vT_sb)
            pg = proj(wq, K, xg2s[g], GW, f"pg_{g}")
            nc.scalar.activation(qT_all[:, ts(g, GW)], pg[:], SIG, bias=bq_ap)

        # ============ sequential phases (v3 schedule) ============
        kn1, v1_b = transposes(kT1, vT1, PCH, "1")
        knv2 = [transposes(kT_all[:, ts(g, GW)], vT_sbs[g], GCH, f"2{g}")
                for g in range(NG)]

        sqs = [scores_stage(g) for g in range(NG)]
        kgeo1 = kgeo_stage(kn1, PCH * 2 * K, wgeo_p1, "1")
        kgeo2s = [kgeo_stage(knv2[g][0], GCH * 2 * K, wgeo_p2, f"2{g}")
                  for g in range(NG)]

        pu1 = pattn.tile([2 * K, V], F32, tag="pA", name="pu1")
        nc.tensor.matmul(pu1[:], kgeo1[:, 0 : 2 * K], v1_b[:, 0:V],
                         start=True, stop=False)
        nc.tensor.matmul(pu1[:], kgeo1[:, 2 * K : 4 * K], v1_b[:, V : 2 * V],
                         start=False, stop=True)
        geo1 = state.tile([2 * K, V], F32, name="geo1")
        nc.vector.tensor_copy(geo1[:], pu1[:])

        pu2 = pattn.tile([2 * K, (NCH - 1) * V], F32, tag="pA", name="pu2")
        for i in range(NCH - 1):
            g, cl = i // GCH, i % GCH
            nc.tensor.matmul(pu2[:, ts(i, V)], kgeo2s[g][:, ts(cl, 2 * K)],
                             knv2[g][1][:, ts(cl, V)], start=True, stop=True)
        nc.vector.tensor_scalar_mul(geo_all[:, 0:V], geo1[:],
                                    gamma[:, G_GAMMA : G_GAMMA + 1])
        nc.gpsimd.tensor_copy(geo_bf[:, 0:V], geo_all[:, 0:V])
        chain_steps(1, NCH, pu2)

        for g in range(NG):
            attn_stage(g, knv2[g][1], sqs[g][0], sqs[g][1])
            out_stage(g)

    nc.compile()
    return nc


_CACHE = {}


def _get_program(has_bv):
    key = ("nc", has_bv)
    if key not in _CACHE:
        _CACHE[key] = _build_program(has_bv)
    return _CACHE[key]


def _make_in_maps(x, Wk, bk, Wv, bv, Wq, bq, Wo):
    import ml_dtypes

    bfd = ml_dtypes.bfloat16
    consts = _host_constants()

    def pack2(Wa, Wb):
        # [128, NEC*(outA+outB)]: per embed sub-chunk, [Wa_ec | Wb_ec] columns
        Wab = np.concatenate(
            [Wa.T.reshape(NEC, 128, -1), Wb.T.reshape(NEC, 128, -1)], 2
        )
        return np.ascontiguousarray(
            Wab.transpose(1, 0, 2).reshape(128, -1)
        ).astype(bfd)

    def pack1(W):
        return np.ascontiguousarray(
            W.T.reshape(NEC, 128, -1).transpose(1, 0, 2).reshape(128, -1)
        ).astype(bfd)

    identhi = np.zeros((128, 64), np.float32)
    identhi[64:128, :] = np.eye(64)
    chot = np.zeros((128, HOT_W), np.float32)

    def setreg(pack, layout, name, arr):
        r, o, c = layout[name]
        pack[0:r, o : o + c] = arr

    setreg(chot, _HOT, "ident", consts["ident64"])
    setreg(chot, _HOT, "identhi", identhi)
    setreg(chot, _HOT, "mlocT4", consts["mlocT4"])
    setreg(chot, _HOT, "cemat", consts["cemat"])
    setreg(chot, _HOT, "comat", consts["comat"])
    setreg(chot, _HOT, "wgeo_p1", consts["wgeo_p1"])
    setreg(chot, _HOT, "wgeo_p2", consts["wgeo_p2"])

    shared = {
        "wproj": np.concatenate([pack2(Wk, Wv), pack1(Wq)], 1),
        "chot": chot.astype(bfd),
        "woT": np.ascontiguousarray(Wo.T).astype(bfd),
    }

    def pack_x(xh):
        # [E, HT] -> [NG*128, (ec, half, t)]: one contiguous block per group
        v = xh.reshape(NEC, 128, NG, 2, HW2).transpose(2, 1, 0, 3, 4)
        return np.ascontiguousarray(v.reshape(NG * 128, NEC * GW)).astype(bfd)

    def pack_pre(xh):
        # last PRE prefix rows -> [128, (ec, t)] contiguous block
        v = xh[:, HT - PRE :].reshape(NEC, 128, PRE).transpose(1, 0, 2)
        return np.ascontiguousarray(v.reshape(128, NEC * PRE)).astype(bfd)

    gcol = np.zeros((128, 3), np.float32)
    gcol[0:K, G_BKV] = bk
    gcol[K : 2 * K, G_BKV] = bv
    gcol[0:K, G_BQ] = bq

    zeros_pre = np.zeros((128, NEC * PRE), bfd)
    in_maps = []
    for c in range(N_CORES):
        b, h = c // 2, c % 2
        xbT = np.ascontiguousarray(x[b].T)  # [E, T]
        m = dict(shared)
        m["xpre2"] = pack_pre(xbT[:, :HT]) if h == 1 else zeros_pre
        m["xq2"] = pack_x(xbT[:, h * HT : (h + 1) * HT])
        g = gcol.copy()
        g[:, G_GAMMA] = float(h)
        m["gamma_col"] = g
        in_maps.append(m)
    return in_maps


def run(inputs, trace=False):
    """Run on 8 cores; returns (output, BassKernelResults)."""
    inp = {k: np.asarray(v) for k, v in inputs.items()}
    has_bv = bool(np.any(inp["bv"]))
    nc = _get_program(has_bv)
    in_maps = _make_in_maps(**inp)
    res = bass_utils.run_bass_kernel_spmd(
        nc, in_maps, core_ids=list(range(N_CORES)), trace=trace
    )
    out = np.empty((B, T, E), np.float32)
    for c in range(N_CORES):
        b, h = c // 2, c % 2
        out[b, h * HT : (h + 1) * HT, :] = res.results[c]["out"].astype(np.float32)
    return out, res


def kernel(**inputs):
    out, _ = run(inputs, trace=False)
    return out


# revision 13
# speedup vs baseline: 1.1350x; 1.0464x over previous
"""Trainium2 Bass kernel for nn_Decay2D (decay-masked linear attention).

Math: the reference's Hillis-Steele scan with decay-squaring order composes
to coefficient d^ceil((t-s)/2) on store[s] = scale*k_s v_s^T, so

    out[t] = scale^2 * sum_{s<=t} d^ceil((t-s)/2) (q_t . k_s) v_s  @ Wo^T

computed as chunked linear attention with two [K,V] carry states (even/odd
decay chains), never materializing the [B,T,K,V] memory.

Sharding: 8 cores = 4 batches x 2 sequence halves. Each core builds the
carry state over its prefix rows and runs full attention + output projection
for its own 1024 rows.

Implementation notes:
- bf16 on the PE, fp32 PSUM accumulation, fp32 carry states.
- Projection biases ride the sigmoid activation's bias AP (no bias matmuls).
- The PE instruction stream is software-pipelined across the two sequence
  groups (g1 transposes/scores overlap g0's state chain and output copies)
  to keep the PE continuously busy and hold the high p-state clock.
- Output is written bf16 (fro tolerance 2e-2) and widened on host; the two
  po halves land in one 2-bank PSUM tile so each chunk needs one wide copy.
"""

from contextlib import ExitStack

import numpy as np

import concourse.bass as bass
import concourse.bacc as bacc
import concourse.mybir as mybir
import concourse.tile as tile
from concourse import bass_utils
from concourse.alu_op_type import AluOpType
from concourse.bass import ts

F32 = mybir.dt.float32
BF16 = mybir.dt.bfloat16
SIG = mybir.ActivationFunctionType.Sigmoid

B, T, E, K, V = 4, 2048, 1024, 64, 64
DECAY = 0.9
C = 128          # chunk length
HT = T // 2      # rows per core (sequence half)
NCH = HT // C    # chunks per half (8)
NEC = E // 128   # embed sub-chunks (8)
GW = 512         # group width: 4 chunks per PSUM bank
GCH = GW // C    # chunks per group (4)
NG = HT // GW    # groups per half (2)
DC2 = float(DECAY ** (C // 2))
N_CORES = 8

# packed-constants layout (masks only; projection weights travel separately)
def _mklayout(regions):
    out, off = {}, 0
    for n, r, c in regions:
        out[n] = (r, off, c)
        off += c
    return out, off


_HOT, HOT_W = _mklayout([
    ("ident", 64, 64), ("identhi", 128, 64),
    ("mlocT4", C, GW), ("cemat", K, GW), ("comat", K, GW),
    ("wgeo_p1", C, GCH * 2 * K), ("wgeo_p2", C, GCH * 2 * K),
])
HW2 = GW // 2   # half-group width
PRE = 256       # truncated prefix length (2 chunks; older rows decay < 1e-12)
PCH = PRE // C  # prefix chunks (2)

# gamma columns: 0 prefix flag, 1 bk|bv, 2 bq
G_GAMMA, G_BKV, G_BQ = 0, 1, 2


def _host_constants():
    d = DECAY
    scale2 = 1.0 - d
    i = np.arange(C)
    j = np.arange(C)
    delta = i[:, None] - j[None, :]
    # intra-chunk decay mask, transposed to [tcol(j), trow(i)], scale^2 folded
    mloc = np.where(delta >= 0, d ** np.ceil(delta / 2.0), 0.0) * scale2
    mlocT4 = np.tile(np.ascontiguousarray(mloc.T), (1, GCH)).astype(np.float32)
    # boundary coefficient per local row i (scale^2 folded), split by parity
    c = d ** np.ceil((i + 1) / 2.0) * scale2
    ce = np.where(i % 2 == 0, c, 0.0).astype(np.float32)
    co = np.where(i % 2 == 1, c, 0.0).astype(np.float32)
    cemat = np.tile(np.broadcast_to(ce, (K, C)), (1, GCH)).astype(np.float32)
    comat = np.tile(np.broadcast_to(co, (K, C)), (1, GCH)).astype(np.float32)
    # state-update row weights (per t within chunk)
    u_o = np.where(j % 2 == 1, d ** ((C - 1 - j) / 2.0), 0.0)
    u_e = np.where(j % 2 == 0, d ** ((C - 2 - j) / 2.0), 0.0)
    wge = (u_o + u_e).astype(np.float32)[:, None]          # [C,1]
    wgo = (u_o + d * u_e).astype(np.float32)[:, None]

    def wmat_stacked(paired):
        # [C, GCH*2K]: per chunk block [wge*K | wgo*K]; paired mode folds an
        # extra dC2 into the first chunk of each pair
        blocks = []
        for cl in range(GCH):
            s = DC2 if (paired and cl % 2 == 0) else 1.0
            blocks.append(np.repeat(np.concatenate([wge * s, wgo * s], 1), K, 1))
        return np.concatenate(blocks, 1).astype(np.float32)

    return {
        "mlocT4": mlocT4,
        "cemat": np.ascontiguousarray(cemat),
        "comat": np.ascontiguousarray(comat),
        "wgeo_p1": wmat_stacked(True),
        "wgeo_p2": wmat_stacked(False),
        "ident64": np.eye(64, dtype=np.float32),
    }


def _build_program(has_bv):
    nc = bacc.Bacc(
        "TRN2",
        debug=False,
        enable_asserts=False,
        target_bir_lowering=False,
        num_devices=N_CORES,
    )

    def din(name, shape, dtype=F32):
        return nc.dram_tensor(name, shape, dtype, kind="ExternalInput").ap()

    # x pre-packed on host: per group one contiguous [128, NEC*GW] block,
    # free layout (ec, half, t). Prefix truncated to its last PRE rows.
    wpx_d = din("wpx", [128, NEC * 3 * K + NEC * PRE], BF16)
    xq2 = din("xq2", [NG * 128, NEC * GW], BF16)
    chot_d = din("chot", [128, HOT_W], BF16)
    woT_d = din("woT", [V, E], BF16)
    gamma_d = din("gamma_col", [128, 3])
    out_d = nc.dram_tensor("out", [NCH // 2, 128, 2 * E], BF16,
                       kind="ExternalOutput").ap()

    with ExitStack() as ctx:
        tc = ctx.enter_context(tile.TileContext(nc))

        consts = ctx.enter_context(tc.tile_pool(name="consts", bufs=1))
        state = ctx.enter_context(tc.tile_pool(name="state", bufs=1))
        xpool = ctx.enter_context(tc.tile_pool(name="xg", bufs=4))
        spool = ctx.enter_context(tc.tile_pool(name="sml", bufs=2))
        opool = ctx.enter_context(tc.tile_pool(name="osb", bufs=3))
        pbig = ctx.enter_context(tc.tile_pool(name="pbig", bufs=2, space="PSUM"))
        psml = ctx.enter_context(tc.tile_pool(name="psml", bufs=2, space="PSUM"))
        pattn = ctx.enter_context(tc.tile_pool(name="pattn", bufs=2, space="PSUM"))

        # ---- loads (issue order = queue order on sync) ----
        def ld(pool, shape, dtype, src, name):
            t = pool.tile(shape, dtype, name=name)
            nc.sync.dma_start(t[:], src)
            return t

        gamma = ld(consts, [128, 3], F32, gamma_d[:], "gamma_sb")
        wpx = ld(consts, [128, NEC * 3 * K + NEC * PRE], BF16, wpx_d[:], "wpx")
        xp = wpx[:, NEC * 3 * K :]

        def load_xgroup(g, name):
            xg = xpool.tile([128, NEC * GW], BF16, tag="xg", name=name)
            nc.sync.dma_start(xg[:], xq2[g * 128 : (g + 1) * 128, :])
            return xg

        xg2_0 = load_xgroup(0, "xg2_0")
        xg2_1 = load_xgroup(1, "xg2_1")
        xg2s = [xg2_0, xg2_1]
        chot = ld(consts, [128, HOT_W], BF16, chot_d[:], "chot")
        wo = ld(consts, [V, E], BF16, woT_d[:], "wo")

        def reg(pack, layout, name):
            r, o, c = layout[name]
            return pack[0:r, o : o + c]

        wkv = wpx[:, 0 : NEC * 2 * K]
        wq = wpx[:, NEC * 2 * K : NEC * 3 * K]
        ident, identhi = reg(chot, _HOT, "ident"), reg(chot, _HOT, "identhi")
        mlocT4 = reg(chot, _HOT, "mlocT4")
        cemat, comat = reg(chot, _HOT, "cemat"), reg(chot, _HOT, "comat")
        wgeo_p1, wgeo_p2 = reg(chot, _HOT, "wgeo_p1"), reg(chot, _HOT, "wgeo_p2")
        bk_ap = gamma[0:K, G_BKV : G_BKV + 1]
        bv_ap = gamma[K : 2 * K, G_BKV : G_BKV + 1]
        bq_ap = gamma[0:K, G_BQ : G_BQ + 1]

        qT_all = consts.tile([K, HT], BF16, name="qT_all")
        kT_all = consts.tile([K, HT], BF16, name="kT_all")
        lt_all = consts.tile([V, HT], BF16, name="lt_all")
        geo_all = state.tile([2 * K, NCH * V], F32, name="geo_all")
        geo_bf = state.tile([2 * K, NCH * V], BF16, name="geo_bf")

        # ---- stage helpers ----
        def proj(w, rows, xg, cols, name):
            p = pbig.tile([rows, cols], F32, tag="pB", name=name)
            for ec in range(NEC):
                nc.tensor.matmul(p[:], w[:, ts(ec, rows)], xg[:, ts(ec, cols)],
                                 start=(ec == 0), stop=(ec == NEC - 1))
            return p

        def transposes(kT_src, vT_src, nch, tagp):
            pkn = psml.tile([C, nch * 2 * K], BF16, tag="pS", name=f"pkn{tagp}")
            for cl in range(nch):
                kT_i = kT_src[:, ts(cl, C)]
                nc.tensor.matmul(pkn[:, cl * 2 * K : cl * 2 * K + K],
                                 kT_i, ident[:], is_transpose=True)
                nc.tensor.matmul(pkn[:, cl * 2 * K + K : (cl + 1) * 2 * K],
                                 kT_i, ident[:], is_transpose=True)
            pvn = psml.tile([C, nch * V], BF16, tag="pS", name=f"pvn{tagp}")
            for cl in range(nch):
                nc.tensor.matmul(pvn[:, ts(cl, V)], vT_src[K : 2 * K, ts(cl, C)],
                                 identhi[K : 2 * K, :], is_transpose=True)
            kn = spool.tile([C, nch * 2 * K], BF16, tag=f"kn{tagp}", name=f"kn{tagp}")
            nc.scalar.copy(kn[:], pkn[:])
            v_b = spool.tile([C, nch * V], BF16, tag=f"v{tagp}", name=f"v{tagp}")
            nc.vector.tensor_copy(v_b[:], pvn[:])
            return kn, v_b

        def scores_stage(g):
            ps = pbig.tile([C, GW], F32, tag="pB", name="ps")
            for cl in range(GCH):
                i = g * GCH + cl
                nc.tensor.matmul(ps[:, ts(cl, C)], kT_all[:, ts(i, C)],
                                 qT_all[:, ts(i, C)], start=True, stop=True)
            sT_b = spool.tile([C, GW], BF16, tag=f"sm{g}", name="sT_b")
            nc.vector.tensor_mul(sT_b[:], ps[:], mlocT4[:])
            qTeo = spool.tile([2 * K, GW], BF16, tag=f"qeo{g}", name="qTeo")
            nc.vector.tensor_mul(qTeo[0:K, :], qT_all[:, ts(g, GW)], cemat[:])
            nc.gpsimd.tensor_mul(qTeo[K : 2 * K, :], qT_all[:, ts(g, GW)], comat[:])
            return sT_b, qTeo

        def kgeo_stage(kn, width, wsel, tagp):
            kgeo = spool.tile([C, width], BF16, tag=f"kg{tagp}", name=f"kgeo{tagp}")
            nc.vector.tensor_mul(kgeo[:], kn[:], wsel[:, :width])
            return kgeo

        def attn_stage(g, v_b, sT_b, qTeo):
            plt = pattn.tile([V, GW], F32, tag="pLT", name="plt")
            for cl in range(GCH):
                i = g * GCH + cl
                dst = plt[:, ts(cl, C)]
                nc.tensor.matmul(dst, v_b[:, ts(cl, V)], sT_b[:, ts(cl, C)],
                                 start=True, stop=False)
                nc.tensor.matmul(dst, geo_bf[:, ts(i, V)], qTeo[:, ts(cl, C)],
                                 start=False, stop=True)
                if cl % 2 == 0:
                    nc.scalar.copy(lt_all[:, ts(i, C)], dst)
                else:
                    nc.vector.tensor_copy(lt_all[:, ts(i, C)], dst)

        def out_stage(g):
            for pr in range(GCH // 2):
                out_sb = opool.tile([C, 2 * E], BF16, tag="osb", name="out_sb")
                for j in range(2):
                    i = g * GCH + pr * 2 + j
                    for h in range(2):
                        po = pbig.tile([C, GW], F32, tag="pB", name="po")
                        nc.tensor.matmul(po[:], lt_all[:, ts(i, C)],
                                         wo[:, ts(h, GW)], start=True, stop=True)
                        dst = out_sb[:, j * E + h * GW : j * E + (h + 1) * GW]
                        if h == 0:
                            nc.scalar.copy(dst, po[:])
                        else:
                            nc.vector.tensor_copy(dst, po[:])
                nc.sync.dma_start(out_d[g * (GCH // 2) + pr], out_sb[:])

        def chain_steps(lo, hi, pu2):
            # geo_i = dC2 * geo_{i-1} + pu2_{i-1}, bf16 copy per chunk
            for i in range(lo, hi):
                nc.vector.scalar_tensor_tensor(
                    geo_all[:, ts(i, V)], geo_all[:, ts(i - 1, V)], DC2,
                    pu2[:, ts(i - 1, V)], AluOpType.mult, AluOpType.add,
                )
                nc.gpsimd.tensor_copy(geo_bf[:, ts(i, V)], geo_all[:, ts(i, V)])

        # ============ projections (PE dense from the start) ============
        pkv1 = proj(wkv, 2 * K, xp, PRE, "pkv1")
        kT1 = spool.tile([K, PRE], BF16, tag="kT1", name="kT1")
        nc.scalar.activation(kT1[:], pkv1[0:K, :], SIG, bias=bk_ap)
        vT1 = spool.tile([2 * K, PRE], BF16, tag="vT1", name="vT1")
        nc.scalar.copy(vT1[K : 2 * K, :], pkv1[K : 2 * K, :])
        if has_bv:
            nc.vector.tensor_scalar_add(vT1[K : 2 * K, :], vT1[K : 2 * K, :], bv_ap)

        vT_sbs = []
        for g in range(NG):
            pkv = proj(wkv, 2 * K, xg2s[g], GW, f"pkv_{g}")
            nc.scalar.activation(kT_all[:, ts(g, GW)], pkv[0:K, :], SIG, bias=bk_ap)
            vT_sb = spool.tile([2 * K, GW], BF16, tag=f"vT{g}", name=f"vT_sb{g}")
            nc.scalar.copy(vT_sb[K : 2 * K, :], pkv[K : 2 * K, :])
            if has_bv:
                nc.vector.tensor_scalar_add(
                    vT_sb[K : 2 * K, :], vT_sb[K : 2 * K, :], bv_ap)
            vT_sbs.append(vT_sb)
            pg = proj(wq, K, xg2s[g], GW, f"pg_{g}")
            nc.scalar.activation(qT_all[:, ts(g, GW)], pg[:], SIG, bias=bq_ap)

        # ============ sequential phases (v3 schedule) ============
        kn1, v1_b = transposes(kT1, vT1, PCH, "1")
        knv2 = [transposes(kT_all[:, ts(g, GW)], vT_sbs[g], GCH, f"2{g}")
                for g in range(NG)]

        sqs = [scores_stage(g) for g in range(NG)]
        kgeo1 = kgeo_stage(kn1, PCH * 2 * K, wgeo_p1, "1")
        kgeo2s = [kgeo_stage(knv2[g][0], GCH * 2 * K, wgeo_p2, f"2{g}")
                  for g in range(NG)]

        pu1 = pattn.tile([2 * K, V], F32, tag="pA", name="pu1")
        nc.tensor.matmul(pu1[:], kgeo1[:, 0 : 2 * K], v1_b[:, 0:V],
                         start=True, stop=False)
        nc.tensor.matmul(pu1[:], kgeo1[:, 2 * K : 4 * K], v1_b[:, V : 2 * V],
                         start=False, stop=True)
        geo1 = state.tile([2 * K, V], F32, name="geo1")
        nc.vector.tensor_copy(geo1[:], pu1[:])

        pu2 = pattn.tile([2 * K, (NCH - 1) * V], F32, tag="pA", name="pu2")
        for i in range(NCH - 1):
            g, cl = i // GCH, i % GCH
            nc.tensor.matmul(pu2[:, ts(i, V)], kgeo2s[g][:, ts(cl, 2 * K)],
                             knv2[g][1][:, ts(cl, V)], start=True, stop=True)
        nc.vector.tensor_scalar_mul(geo_all[:, 0:V], geo1[:],
                                    gamma[:, G_GAMMA : G_GAMMA + 1])
        nc.gpsimd.tensor_copy(geo_bf[:, 0:V], geo_all[:, 0:V])
        chain_steps(1, NCH, pu2)

        for g in range(NG):
            attn_stage(g, knv2[g][1], sqs[g][0], sqs[g][1])
            out_stage(g)

    nc.compile()
    return nc


_CACHE = {}


def _get_program(has_bv):
    key = ("nc", has_bv)
    if key not in _CACHE:
        _CACHE[key] = _build_program(has_bv)
    return _CACHE[key]


def _make_in_maps(x, Wk, bk, Wv, bv, Wq, bq, Wo):
    import ml_dtypes

    bfd = ml_dtypes.bfloat16
    consts = _host_constants()

    def pack2(Wa, Wb):
        # [128, NEC*(outA+outB)]: per embed sub-chunk, [Wa_ec | Wb_ec] columns
        Wab = np.concatenate(
            [Wa.T.reshape(NEC, 128, -1), Wb.T.reshape(NEC, 128, -1)], 2
        )
        return np.ascontiguousarray(
            Wab.transpose(1, 0, 2).reshape(128, -1)
        ).astype(bfd)

    def pack1(W):
        return np.ascontiguousarray(
            W.T.reshape(NEC, 128, -1).transpose(1, 0, 2).reshape(128, -1)
        ).astype(bfd)

    identhi = np.zeros((128, 64), np.float32)
    identhi[64:128, :] = np.eye(64)
    chot = np.zeros((128, HOT_W), np.float32)

    def setreg(pack, layout, name, arr):
        r, o, c = layout[name]
        pack[0:r, o : o + c] = arr

    setreg(chot, _HOT, "ident", consts["ident64"])
    setreg(chot, _HOT, "identhi", identhi)
    setreg(chot, _HOT, "mlocT4", consts["mlocT4"])
    setreg(chot, _HOT, "cemat", consts["cemat"])
    setreg(chot, _HOT, "comat", consts["comat"])
    setreg(chot, _HOT, "wgeo_p1", consts["wgeo_p1"])
    setreg(chot, _HOT, "wgeo_p2", consts["wgeo_p2"])

    wproj_pack = np.concatenate([pack2(Wk, Wv), pack1(Wq)], 1)
    shared = {
        "chot": chot.astype(bfd),
        "woT": np.ascontiguousarray(Wo.T).astype(bfd),
    }

    def pack_x(xh):
        # [E, HT] -> [NG*128, (ec, half, t)]: one contiguous block per group
        v = xh.reshape(NEC, 128, NG, 2, HW2).transpose(2, 1, 0, 3, 4)
        return np.ascontiguousarray(v.reshape(NG * 128, NEC * GW)).astype(bfd)

    def pack_pre(xh):
        # last PRE prefix rows -> [128, (ec, t)] contiguous block
        v = xh[:, HT - PRE :].reshape(NEC, 128, PRE).transpose(1, 0, 2)
        return np.ascontiguousarray(v.reshape(128, NEC * PRE)).astype(bfd)

    gcol = np.zeros((128, 3), np.float32)
    gcol[0:K, G_BKV] = bk
    gcol[K : 2 * K, G_BKV] = bv
    gcol[0:K, G_BQ] = bq

    zeros_pre = np.zeros((128, NEC * PRE), bfd)
    in_maps = []
    for c in range(N_CORES):
        b, h = c // 2, c % 2
        xbT = np.ascontiguousarray(x[b].T)  # [E, T]
        m = dict(shared)
        pre = pack_pre(xbT[:, :HT]) if h == 1 else zeros_pre
        m["wpx"] = np.ascontiguousarray(np.concatenate([wproj_pack, pre], 1))
        m["xq2"] = pack_x(xbT[:, h * HT : (h + 1) * HT])
        g = gcol.copy()
        g[:, G_GAMMA] = float(h)
        m["gamma_col"] = g
        in_maps.append(m)
    return in_maps


def run(inputs, trace=False):
    """Run on 8 cores; returns (output, BassKernelResults)."""
    inp = {k: np.asarray(v) for k, v in inputs.items()}
    has_bv = bool(np.any(inp["bv"]))
    nc = _get_program(has_bv)
    in_maps = _make_in_maps(**inp)
    res = bass_utils.run_bass_kernel_spmd(
        nc, in_maps, core_ids=list(range(N_CORES)), trace=trace
    )
    out = np.empty((B, T, E), np.float32)
    for c in range(N_CORES):
        b, h = c // 2, c % 2
        o = res.results[c]["out"].astype(np.float32)
        o = o.reshape(NCH // 2, 128, 2, E).transpose(0, 2, 1, 3).reshape(HT, E)
        out[b, h * HT : (h + 1) * HT, :] = o
    return out, res


def kernel(**inputs):
    out, _ = run(inputs, trace=False)
    return out
